# revision 15
# baseline (speedup 1.0000x reference)
"""Trainium2 Bass kernel for nn_ATT_critic (attention critic network).

Strategy: data-parallel over batch across 8 NeuronCores (1024 rows/core).
All large GEMMs run on the PE in fp32r (full rate at N=512 free dim).

Per-core dataflow (2 chunks of 512 rows):
  - s/a are PE-transposed (identity matmul) into [feat, rows] layout.
  - enc_input^T / encoder_h^T / x1^T computed "transposed-out"
    (lhsT=W tile, rhs=X^T)  -> feature on partitions, bias via per-partition
    ACT bias, relu fused into the PSUM->SBUF eviction on the scalar engine.
  - decoder path is algebraically collapsed: dec_input feeds only decoder_H,
    so W_fused = W_dec_in @ W_dh and b_fused = b_dec_in @ W_dh + b_dh are
    precomputed on-device once; decoder_H = relu(a_others @ W_fused + b_fused)
    is emitted row-major (lhsT = a_others^T, rhs = W_fused) with the bias
    added by a K=1 ones-row matmul.
  - heads are emitted row-major (lhsT = encoder_h^T tile, rhs = W_h) so that
    scores  = rowdot(EH_h, DH): DVE multiply + ACT Copy-with-accum rowsum
    softmax -> tiny per-partition ops (rows on partitions)
    context = sum_h attn_h * EH_h -> fused DVE scalar_tensor_tensor, bf16
  - context is PE-transposed (bf16) back to [feat, rows] for fc1; fc2 is a
    thin M=1 transposed-out matmul producing q^T [1, rows] directly.

EH (8 heads x 512 rows x 1024) is stored bf16 to fit SBUF; weights stream
through a double-buffered pool of 16KB/partition fp32r tiles (M-halves for
the transposed-out layers, N-halves for the row-major heads layer). The
W_dh halves for the fused-decoder precompute ride through the t8 activation
slots so their DMA starts at t=0 without blocking the weight pool.
"""

import numpy as np

import concourse.bass as bass
import concourse.tile as tile
from concourse import mybir
from concourse import bacc
from concourse.masks import make_identity

P = 128
B = 8192
NCORES = 8
RPC = B // NCORES        # rows per core
CH = 512                 # rows per chunk
NCHUNK = RPC // CH
MT = CH // P             # row tiles per chunk
HID = 1024
KT = HID // P            # k tiles over hidden dim
NH = 8                   # heads
OBS4 = 512               # n_agents*obs
ACTD = 32
DEC_IN = 96
ENC_REM = 32             # 544 - 512

F32 = mybir.dt.float32
F32R = mybir.dt.float32r
BF16 = mybir.dt.bfloat16
AF = mybir.ActivationFunctionType
ALU = mybir.AluOpType
AX = mybir.AxisListType

WEIGHT_NAMES = [
    "W_enc_in", "b_enc_in", "W_dec_in", "b_dec_in", "W_eh", "b_eh",
    "W_heads", "b_heads", "W_dh", "b_dh", "W1", "b1", "W2", "b2",
]


def _r(ap):
    return ap.bitcast(F32R)


def _body(nc, tc, io, ctx):
    s_ap = io["s"]
    a_ap = io["a"]
    q_ap = io["q"]

    const = ctx.enter_context(tc.tile_pool(name="const", bufs=1))
    acts = ctx.enter_context(tc.tile_pool(name="acts", bufs=1))
    wp = ctx.enter_context(tc.tile_pool(name="wp", bufs=2))
    ps = ctx.enter_context(tc.tile_pool(name="ps", bufs=1, space="PSUM"))

    # tag helpers: every tile() for a tag must pass the same bufs
    def wtile(shape, name, dtype=F32R):
        return wp.tile(shape, dtype, tag="w", bufs=2, name=name)

    def t8tile(shape, name, dtype=F32R):
        return acts.tile(shape, dtype, tag="t8", bufs=3, name=name)

    def junk(shape, dtype, name):
        return acts.tile(shape, dtype, tag="junk", bufs=2, name=name)

    def psmm(name, shape=None):
        return ps.tile(shape or [P, 512], F32, tag="mm", bufs=4, name=name)

    def pstr(name, dtype=F32):
        return ps.tile([P, 512], dtype, tag="tr", bufs=2, name=name)

    def psq(name):
        return ps.tile([1, 512], F32, tag="q", bufs=2, name=name)

    # ---------------- constants / one-time init ----------------
    identity = const.tile([P, P], F32, name="identity")
    make_identity(nc, identity)
    identity_bf = const.tile([P, P], BF16, name="identity_bf")
    nc.vector.tensor_copy(identity_bf, identity)
    ones_bf = const.tile([1, P], BF16, name="ones_bf")
    nc.vector.memset(ones_bf, 1.0)

    b_enc_pp = const.tile([P, KT], F32, name="b_enc_pp")
    nc.sync.dma_start(b_enc_pp, io["b_enc_in"].rearrange("(o p) -> p o", p=P))
    b_eh_pp = const.tile([P, KT], F32, name="b_eh_pp")
    nc.sync.dma_start(b_eh_pp, io["b_eh"].rearrange("(o p) -> p o", p=P))
    b1_pp = const.tile([P, KT], F32, name="b1_pp")
    nc.sync.dma_start(b1_pp, io["b1"].rearrange("(o p) -> p o", p=P))
    bdec_pp = const.tile([P, KT], F32R, name="bdec_pp")
    nc.sync.dma_start(bdec_pp, io["b_dec_in"].rearrange("(o p) -> p o", p=P).bitcast(F32R))
    W2sb = const.tile([P, KT], F32R, name="W2sb")
    nc.sync.dma_start(W2sb, io["W2"].rearrange("(o p) one -> p (o one)", p=P).bitcast(F32R))
    b2sb = const.tile([1, 1], F32, name="b2sb")
    nc.sync.dma_start(b2sb, io["b2"][None, :])
    # enc remainder rows (a_own part of W_enc): loaded once, reused by chunks
    wencr = const.tile([ENC_REM, HID], F32R, name="wencr")
    nc.sync.dma_start(wencr, io["W_enc_in"][512:544, :].bitcast(F32R))

    # bias rows (bf16, partition 0) for the K=1 ones-row bias matmuls
    bh_row = const.tile([1, NH, HID], BF16, name="bh_row")
    for h in range(NH):
        for half in range(2):
            jt = junk([P, HID], BF16, "jstage")
            jf = jt.bitcast(F32)  # [P, 512] f32 view
            nc.sync.dma_start(jf[0:1, :],
                              io["b_heads"][h, half * 512:(half + 1) * 512][None, :])
            nc.vector.tensor_copy(bh_row[0:1, h, half * 512:(half + 1) * 512],
                                  jf[0:1, :])

    # W_dh halves for the fused-decoder precompute: DMA starts immediately,
    # parked in two t8 slots (freed after the precompute matmuls).
    wdh_halves = []
    for half in range(2):
        wdh = t8tile([P, KT // 2, HID], f"wdh{half}")
        nc.sync.dma_start(
            wdh,
            io["W_dh"][half * 512:(half + 1) * 512, :].rearrange(
                "(ko p) f -> p ko f", p=P).bitcast(F32R))
        wdh_halves.append(wdh)

    wdecT = const.tile([P, KT, DEC_IN], F32R, name="wdecT")
    wfused = const.tile([DEC_IN, HID], F32R, name="wfused")
    bfused_row = const.tile([1, HID], BF16, name="bfused_row")

    def emit_precompute():
        # W_fused[96,1024] = W_dec_in @ W_dh ; b_fused = b_dec_in @ W_dh + b_dh
        wdec = wtile([DEC_IN, HID], "wdec", dtype=F32)
        nc.sync.dma_start(wdec, io["W_dec_in"])
        for kt in range(KT):
            trp = pstr("trw")
            nc.tensor.transpose(trp[:, 0:DEC_IN], wdec[:, kt * P:(kt + 1) * P],
                                identity[0:DEC_IN, 0:DEC_IN])
            nc.vector.tensor_copy(wdecT[:, kt, :], trp[:, 0:DEC_IN])
        pW = [psmm(f"pW{n}", [DEC_IN, 512]) for n in range(2)]
        pB = [psq(f"pB{n}") for n in range(2)]
        for half in range(2):
            wdh = wdh_halves[half]
            for ko in range(KT // 2):
                kt = half * (KT // 2) + ko
                for n in range(2):
                    nc.tensor.matmul(pW[n], lhsT=_r(wdecT[:, kt, :]),
                                     rhs=_r(wdh[:, ko, n * 512:(n + 1) * 512]),
                                     start=(kt == 0), stop=(kt == KT - 1))
                    nc.tensor.matmul(pB[n], lhsT=_r(bdec_pp[:, kt:kt + 1]),
                                     rhs=_r(wdh[:, ko, n * 512:(n + 1) * 512]),
                                     start=(kt == 0), stop=(kt == KT - 1))
        for n in range(2):
            nc.vector.tensor_copy(wfused[:, n * 512:(n + 1) * 512], pW[n])
            jt = junk([P, HID], BF16, "jstage2")
            jf = jt.bitcast(F32)
            nc.sync.dma_start(jf[0:1, :], io["b_dh"][n * 512:(n + 1) * 512][None, :])
            nc.vector.tensor_tensor(bfused_row[0:1, n * 512:(n + 1) * 512],
                                    pB[n][0:1, :], jf[0:1, :], ALU.add)

    # ---------------- per-chunk main pipeline ----------------
    _scope_stack = []
    c = 0

    def sc_(nm):
        if _scope_stack:
            prev_nm, prev_id = _scope_stack.pop()
            nc.leave_named_scope(prev_nm, prev_id, False)
        if nm is not None:
            full = f"c{c}_{nm}"
            sid, _ = nc.enter_named_scope(full, False)
            _scope_stack.append((full, sid))

    for c in range(NCHUNK):
        r0 = c * CH

        sc_("p1_transp")
        # --- phase 1a: load + transpose s and a ---
        a_rm = junk([P, MT, P], F32, "a_rm")
        nc.sync.dma_start(a_rm,
                          a_ap[r0:r0 + CH, :].rearrange("(rt p) f -> p rt f", p=P))

        sT = acts.tile([P, 4, CH], F32R, tag="sT", bufs=1, name="sT")
        aownT = acts.tile([ACTD, CH], F32R, tag="aownT", bufs=1, name="aownT")
        aothT = acts.tile([DEC_IN, CH], F32R, tag="aothT", bufs=1, name="aothT")
        for rh in range(2):
            s_rm = acts.tile([P, 2, OBS4], F32, tag="s_rm", bufs=1, name="s_rm")
            nc.sync.dma_start(
                s_rm,
                s_ap[r0 + rh * 256:r0 + (rh + 1) * 256, :].rearrange(
                    "(rt p) f -> p rt f", p=P))
            for ri in range(2):
                rt = rh * 2 + ri
                trp = pstr("trs")
                for ft in range(4):
                    nc.tensor.transpose(trp[:, ft * P:(ft + 1) * P],
                                        s_rm[:, ri, ft * P:(ft + 1) * P], identity)
                nc.vector.tensor_copy(sT[:, :, rt * P:(rt + 1) * P],
                                      trp.rearrange("p (ft x) -> p ft x", ft=4))
        for rt in range(MT):
            trp = pstr("tra")
            nc.tensor.transpose(trp[0:ACTD, 0:P], a_rm[:, rt, 0:ACTD], identity)
            nc.tensor.transpose(trp[0:DEC_IN, P:2 * P], a_rm[:, rt, ACTD:P], identity)
            nc.vector.tensor_copy(aownT[:, rt * P:(rt + 1) * P], trp[0:ACTD, 0:P])
            nc.vector.tensor_copy(aothT[:, rt * P:(rt + 1) * P],
                                  trp[0:DEC_IN, P:2 * P])

        sc_("p1b_enc")
        # --- phase 1b: enc_input^T = W_enc^T [s; a_own]^T + b  (no relu) ---
        wenc = wtile([P, 4, HID], "wenc")
        nc.sync.dma_start(
            wenc,
            io["W_enc_in"][0:512, :].rearrange("(ko p) f -> p ko f", p=P).bitcast(F32R))

        enc_inT = t8tile([P, KT, CH], "enc_inT")
        for m in range(KT):
            pm = psmm("pm_enc")
            for kt in range(4):
                nc.tensor.matmul(pm, lhsT=wenc[:, kt, m * P:(m + 1) * P],
                                 rhs=sT[:, kt, :], start=(kt == 0), stop=False)
            nc.tensor.matmul(pm, lhsT=wencr[:, m * P:(m + 1) * P],
                             rhs=aownT, start=False, stop=True)
            nc.scalar.activation(enc_inT[:, m, :], pm, AF.Identity,
                                 bias=b_enc_pp[:, m:m + 1])

        if c == 0:
            sc_("precompute")
            emit_precompute()

        sc_("p2a_eh")
        # --- phase 2a: encoder_h^T = relu(W_eh^T enc_input^T + b_eh) ---
        encHT = t8tile([P, KT, CH], "encHT")
        for mh in range(2):
            weh = wtile([P, KT, 512], "weh")
            nc.sync.dma_start(
                weh,
                io["W_eh"][:, mh * 512:(mh + 1) * 512].rearrange(
                    "(ko p) f -> p ko f", p=P).bitcast(F32R))
            for mi in range(4):
                m = mh * 4 + mi
                pm = psmm("pm_eh")
                for kt in range(KT):
                    nc.tensor.matmul(pm, lhsT=weh[:, kt, mi * P:(mi + 1) * P],
                                     rhs=enc_inT[:, kt, :],
                                     start=(kt == 0), stop=(kt == KT - 1))
                nc.scalar.activation(encHT[:, m, :], pm, AF.Relu,
                                     bias=b_eh_pp[:, m:m + 1])

        sc_("p2b_dh")
        # --- phase 2b: decoder_H = relu(a_others @ W_fused + b_fused), row-major ---
        DH = acts.tile([P, MT, HID], BF16, tag="dh", bufs=1, name="DH")
        for mt in range(MT):
            for n in range(2):
                pm = psmm("pm_dh")
                nc.tensor.matmul(pm, lhsT=aothT[:, mt * P:(mt + 1) * P],
                                 rhs=wfused[:, n * 512:(n + 1) * 512],
                                 start=True, stop=False)
                nc.tensor.matmul(pm, lhsT=ones_bf,
                                 rhs=bfused_row[0:1, n * 512:(n + 1) * 512],
                                 start=False, stop=True)
                nc.scalar.activation(DH[:, mt, n * 512:(n + 1) * 512], pm, AF.Relu)

        sc_("p3_heads")
        # --- phase 3: heads (row-major) + scores ---
        EH = acts.tile([P, MT, NH, HID], BF16, tag="eh", bufs=1, name="EH")
        scores = acts.tile([P, MT, NH], F32, tag="scores", bufs=2, name="scores")

        for h in range(NH):
            for n in range(2):
                whn = wtile([P, KT, 512], f"whn{h}_{n}")
                nc.sync.dma_start(
                    whn,
                    io["W_heads"][h][:, n * 512:(n + 1) * 512].rearrange(
                        "(ko p) f -> p ko f", p=P).bitcast(F32R))
                for mt in range(MT):
                    pm = psmm("pm_hd")
                    for kt in range(KT):
                        nc.tensor.matmul(
                            pm, lhsT=encHT[:, kt, mt * P:(mt + 1) * P],
                            rhs=whn[:, kt, :], start=(kt == 0), stop=False)
                    nc.tensor.matmul(pm, lhsT=ones_bf,
                                     rhs=bh_row[0:1, h, n * 512:(n + 1) * 512],
                                     start=False, stop=True)
                    nc.scalar.activation(EH[:, mt, h, n * 512:(n + 1) * 512],
                                         pm, AF.Relu)
            for mt in range(MT):
                # scores[:, mt, h] = rowsum(EH_h * DH): DVE multiply, then a
                # free-dim sum (alternating DVE/ACT to balance engine load).
                jt = junk([P, HID], BF16, "jsc")
                nc.vector.tensor_tensor(jt[:, :], EH[:, mt, h, :], DH[:, mt, :],
                                        ALU.mult)
                if mt % 2 == 0:
                    nc.scalar.activation(jt[:, :], jt[:, :], AF.Copy,
                                         accum_out=scores[:, mt, h:h + 1])
                else:
                    nc.vector.tensor_reduce(scores[:, mt, h:h + 1], jt[:, :],
                                            axis=AX.X, op=ALU.add)

        sc_("p4_attn")
        # --- phase 4: softmax over heads (scores bounded -> no max-sub) and
        # context accumulation: ACT does the per-partition scale-copies while
        # DVE chains the adds, so the two engines pipeline the tail. ---
        attn = acts.tile([P, MT, NH], F32, tag="attn", bufs=2, name="attn")
        stats = acts.tile([P, MT, 2], F32, tag="stats", bufs=2, name="stats")
        ctx_t = acts.tile([P, MT, HID], BF16, tag="dh", bufs=1, name="ctx_t")
        for mt in range(MT):
            sumexp = stats[:, mt, 0:1]
            rsum = stats[:, mt, 1:2]
            nc.scalar.activation(attn[:, mt, :], scores[:, mt, :], AF.Exp,
                                 accum_out=sumexp)
            nc.vector.reciprocal(rsum, sumexp)
            nc.vector.tensor_scalar_mul(attn[:, mt, :], attn[:, mt, :], rsum)
        for mt in range(MT):
            nc.vector.tensor_scalar_mul(ctx_t[:, mt, :], EH[:, mt, 0, :],
                                        attn[:, mt, 0:1])
            for h in range(1, NH):
                jt = junk([P, HID], BF16, "jctx")
                nc.scalar.activation(jt[:, :], EH[:, mt, h, :], AF.Copy,
                                     scale=attn[:, mt, h:h + 1])
                nc.vector.tensor_tensor(ctx_t[:, mt, :], ctx_t[:, mt, :],
                                        jt[:, :], ALU.add)

        sc_("p5_ctxT")
        # --- phase 5: transpose ctx back to [feat, rows] (bf16 transposes) ---
        ctxT = t8tile([P, KT, CH], "ctxT")
        for mt in range(MT):
            for g in range(2):
                trp = pstr("trc", dtype=BF16)
                for ft in range(4):
                    nc.tensor.transpose(
                        trp[:, ft * P:(ft + 1) * P],
                        ctx_t[:, mt, (g * 4 + ft) * P:(g * 4 + ft + 1) * P],
                        identity_bf)
                nc.vector.tensor_copy(
                    ctxT[:, g * 4:(g + 1) * 4, mt * P:(mt + 1) * P],
                    trp.rearrange("p (ft x) -> p ft x", ft=4))

        sc_("p6_fc1")
        # --- phase 6: x1^T = relu(W1^T ctx^T + b1) ---
        x1T = t8tile([P, KT, CH], "x1T")
        for mh in range(2):
            w1 = wtile([P, KT, 512], "w1t")
            nc.sync.dma_start(
                w1,
                io["W1"][:, mh * 512:(mh + 1) * 512].rearrange(
                    "(ko p) f -> p ko f", p=P).bitcast(F32R))
            for mi in range(4):
                m = mh * 4 + mi
                pm = psmm("pm_fc1")
                for kt in range(KT):
                    nc.tensor.matmul(pm, lhsT=w1[:, kt, mi * P:(mi + 1) * P],
                                     rhs=ctxT[:, kt, :],
                                     start=(kt == 0), stop=(kt == KT - 1))
                nc.scalar.activation(x1T[:, m, :], pm, AF.Relu,
                                     bias=b1_pp[:, m:m + 1])

        sc_("p7_fc2")
        # --- phase 7: q^T = W2^T x1^T + b2  -> [1, CH] ---
        pq = psq("pq")
        for kt in range(KT):
            nc.tensor.matmul(pq, lhsT=W2sb[:, kt:kt + 1], rhs=x1T[:, kt, :],
                             start=(kt == 0), stop=(kt == KT - 1))
        q_rowT = acts.tile([1, CH], F32, tag="q_rowT", bufs=1, name="q_rowT")
        nc.scalar.activation(q_rowT[0:1, :], pq[0:1, :], AF.Identity,
                             bias=b2sb[0:1, 0:1])
        nc.sync.dma_start(q_ap[r0:r0 + CH, 0][None, :], q_rowT[0:1, :])
        sc_(None)


_NC_CACHE = None


def build():
    global _NC_CACHE
    if _NC_CACHE is not None:
        return _NC_CACHE
    nc = bacc.Bacc(trn_type="TRN2", target_bir_lowering=False, debug=False,
                   enable_asserts=False)
    io = {}
    io["s"] = nc.dram_tensor("s", [RPC, 512], F32, kind="ExternalInput").ap()
    io["a"] = nc.dram_tensor("a", [RPC, 128], F32, kind="ExternalInput").ap()
    shapes = {
        "W_enc_in": [544, HID], "b_enc_in": [HID],
        "W_dec_in": [DEC_IN, HID], "b_dec_in": [HID],
        "W_eh": [HID, HID], "b_eh": [HID],
        "W_heads": [NH, HID, HID], "b_heads": [NH, HID],
        "W_dh": [HID, HID], "b_dh": [HID],
        "W1": [HID, HID], "b1": [HID],
        "W2": [HID, 1], "b2": [1],
    }
    for name, shp in shapes.items():
        io[name] = nc.dram_tensor(name, shp, F32, kind="ExternalInput").ap()
    io["q"] = nc.dram_tensor("q", [RPC, 1], F32, kind="ExternalOutput").ap()

    from contextlib import ExitStack
    with tile.TileContext(nc) as tc, ExitStack() as ctx:
        _body(nc, tc, io, ctx)
    nc.compile()
    _NC_CACHE = nc
    return nc


def _shard_inputs(inputs):
    arrs = {k: np.ascontiguousarray(np.asarray(v, dtype=np.float32))
            for k, v in inputs.items()}
    in_maps = []
    for c in range(NCORES):
        m = {k: arrs[k] for k in WEIGHT_NAMES}
        m["s"] = np.ascontiguousarray(arrs["s"][c * RPC:(c + 1) * RPC])
        m["a"] = np.ascontiguousarray(arrs["a"][c * RPC:(c + 1) * RPC])
        in_maps.append(m)
    return in_maps


def run(inputs, trace=False):
    from concourse.bass_utils import run_bass_kernel_spmd
    nc = build()
    in_maps = _shard_inputs(inputs)
    res = run_bass_kernel_spmd(nc, in_maps, core_ids=list(range(NCORES)),
                               trace=trace)
    q = np.concatenate([r["q"] for r in res.results], axis=0)
    return np.ascontiguousarray(q.astype(np.float32)), res


def kernel(**inputs) -> np.ndarray:
    q, _ = run(inputs, trace=False)
    return q


# revision 16
# speedup vs baseline: 1.1439x; 1.1439x over previous
"""Trainium2 Bass kernel for nn_ATT_critic (attention critic network).

Strategy: data-parallel over batch across 8 NeuronCores (1024 rows/core).
All large GEMMs run on the PE in fp32r (full rate at N=512 free dim).

Per-core dataflow (2 chunks of 512 rows):
  - s/a are PE-transposed (identity matmul) into [feat, rows] layout.
  - enc_input^T / encoder_h^T / x1^T computed "transposed-out"
    (lhsT=W tile, rhs=X^T)  -> feature on partitions, bias via per-partition
    ACT bias, relu fused into the PSUM->SBUF eviction on the scalar engine.
  - decoder path is algebraically collapsed: dec_input feeds only decoder_H,
    so W_fused = W_dec_in @ W_dh and b_fused = b_dec_in @ W_dh + b_dh are
    precomputed on-device once; decoder_H = relu(a_others @ W_fused + b_fused)
    is emitted row-major (lhsT = a_others^T, rhs = W_fused) with the bias
    added by a K=1 ones-row matmul.
  - heads are emitted row-major (lhsT = encoder_h^T tile, rhs = W_h) so that
    scores  = rowdot(EH_h, DH): DVE multiply + ACT Copy-with-accum rowsum
    softmax -> tiny per-partition ops (rows on partitions)
    context = sum_h attn_h * EH_h -> fused DVE scalar_tensor_tensor, bf16
  - context is PE-transposed (bf16) back to [feat, rows] for fc1; fc2 is a
    thin M=1 transposed-out matmul producing q^T [1, rows] directly.

EH (8 heads x 512 rows x 1024) is stored bf16 to fit SBUF; weights stream
through a double-buffered pool of 16KB/partition fp32r tiles (M-halves for
the transposed-out layers, N-halves for the row-major heads layer). The
W_dh halves for the fused-decoder precompute ride through the t8 activation
slots so their DMA starts at t=0 without blocking the weight pool.
"""

import numpy as np

import concourse.bass as bass
import concourse.tile as tile
from concourse import mybir
from concourse import bacc
from concourse.masks import make_identity

P = 128
B = 8192
NCORES = 8
RPC = B // NCORES        # rows per core
CH = 512                 # rows per chunk
NCHUNK = RPC // CH
MT = CH // P             # row tiles per chunk
HID = 1024
KT = HID // P            # k tiles over hidden dim
NH = 8                   # heads
OBS4 = 512               # n_agents*obs
ACTD = 32
DEC_IN = 96
ENC_REM = 32             # 544 - 512

F32 = mybir.dt.float32
F32R = mybir.dt.float32r
BF16 = mybir.dt.bfloat16
AF = mybir.ActivationFunctionType
ALU = mybir.AluOpType
AX = mybir.AxisListType

WEIGHT_NAMES = [
    "W_enc_in", "b_enc_in", "W_dec_in", "b_dec_in", "W_eh", "b_eh",
    "W_heads", "b_heads", "W_dh", "b_dh", "W1", "b1", "W2", "b2",
]


def _r(ap):
    return ap.bitcast(F32R)


def _body(nc, tc, io, ctx):
    s_ap = io["s"]
    a_ap = io["a"]
    q_ap = io["q"]

    const = ctx.enter_context(tc.tile_pool(name="const", bufs=1))
    acts = ctx.enter_context(tc.tile_pool(name="acts", bufs=1))
    wp = ctx.enter_context(tc.tile_pool(name="wp", bufs=2))
    ps = ctx.enter_context(tc.tile_pool(name="ps", bufs=1, space="PSUM"))

    # tag helpers: every tile() for a tag must pass the same bufs
    def wtile(shape, name, dtype=F32R):
        return wp.tile(shape, dtype, tag="w", bufs=2, name=name)

    def t8tile(shape, name, dtype=F32R):
        return acts.tile(shape, dtype, tag="t8", bufs=3, name=name)

    def junk(shape, dtype, name):
        return acts.tile(shape, dtype, tag="junk", bufs=2, name=name)

    def psmm(name, shape=None):
        return ps.tile(shape or [P, 512], F32, tag="mm", bufs=4, name=name)

    def pstr(name, dtype=F32):
        return ps.tile([P, 512], dtype, tag="tr", bufs=2, name=name)

    def psq(name):
        return ps.tile([1, 512], F32, tag="q", bufs=2, name=name)

    # ---------------- constants / one-time init ----------------
    identity = const.tile([P, P], F32, name="identity")
    make_identity(nc, identity)
    identity_bf = const.tile([P, P], BF16, name="identity_bf")
    nc.vector.tensor_copy(identity_bf, identity)
    ones_bf = const.tile([1, P], BF16, name="ones_bf")
    nc.vector.memset(ones_bf, 1.0)

    b_enc_pp = const.tile([P, KT], F32, name="b_enc_pp")
    nc.sync.dma_start(b_enc_pp, io["b_enc_in"].rearrange("(o p) -> p o", p=P))
    b_eh_pp = const.tile([P, KT], F32, name="b_eh_pp")
    nc.sync.dma_start(b_eh_pp, io["b_eh"].rearrange("(o p) -> p o", p=P))
    b1_pp = const.tile([P, KT], F32, name="b1_pp")
    nc.sync.dma_start(b1_pp, io["b1"].rearrange("(o p) -> p o", p=P))
    bdec_pp = const.tile([P, KT], F32R, name="bdec_pp")
    nc.sync.dma_start(bdec_pp, io["b_dec_in"].rearrange("(o p) -> p o", p=P).bitcast(F32R))
    W2sb = const.tile([P, KT], F32R, name="W2sb")
    nc.sync.dma_start(W2sb, io["W2"].rearrange("(o p) one -> p (o one)", p=P).bitcast(F32R))
    b2sb = const.tile([1, 1], F32, name="b2sb")
    nc.sync.dma_start(b2sb, io["b2"][None, :])
    # enc remainder rows (a_own part of W_enc): loaded once, reused by chunks
    wencr = const.tile([ENC_REM, HID], F32R, name="wencr")
    nc.sync.dma_start(wencr, io["W_enc_in"][512:544, :].bitcast(F32R))

    # bias rows (bf16, partition 0) for the K=1 ones-row bias matmuls
    bh_row = const.tile([1, NH, HID], BF16, name="bh_row")
    for h in range(NH):
        for half in range(2):
            jt = junk([P, HID], BF16, "jstage")
            jf = jt.bitcast(F32)  # [P, 512] f32 view
            nc.sync.dma_start(jf[0:1, :],
                              io["b_heads"][h, half * 512:(half + 1) * 512][None, :])
            nc.vector.tensor_copy(bh_row[0:1, h, half * 512:(half + 1) * 512],
                                  jf[0:1, :])

    # W_dh halves for the fused-decoder precompute: DMA starts immediately,
    # parked in two t8 slots (freed after the precompute matmuls).
    wdh_halves = []
    for half in range(2):
        wdh = t8tile([P, KT // 2, HID], f"wdh{half}")
        nc.sync.dma_start(
            wdh,
            io["W_dh"][half * 512:(half + 1) * 512, :].rearrange(
                "(ko p) f -> p ko f", p=P).bitcast(F32R))
        wdh_halves.append(wdh)

    wdecT = const.tile([P, KT, DEC_IN], F32R, name="wdecT")
    wfused = const.tile([DEC_IN, HID], F32R, name="wfused")
    bfused_row = const.tile([1, HID], BF16, name="bfused_row")

    def emit_precompute():
        # W_fused[96,1024] = W_dec_in @ W_dh ; b_fused = b_dec_in @ W_dh + b_dh
        wdec = wtile([DEC_IN, HID], "wdec", dtype=F32)
        nc.sync.dma_start(wdec, io["W_dec_in"])
        for kt in range(KT):
            trp = pstr("trw")
            nc.tensor.transpose(trp[:, 0:DEC_IN], wdec[:, kt * P:(kt + 1) * P],
                                identity[0:DEC_IN, 0:DEC_IN])
            nc.vector.tensor_copy(wdecT[:, kt, :], trp[:, 0:DEC_IN])
        pW = [psmm(f"pW{n}", [DEC_IN, 512]) for n in range(2)]
        pB = [psq(f"pB{n}") for n in range(2)]
        for half in range(2):
            wdh = wdh_halves[half]
            for ko in range(KT // 2):
                kt = half * (KT // 2) + ko
                for n in range(2):
                    nc.tensor.matmul(pW[n], lhsT=_r(wdecT[:, kt, :]),
                                     rhs=_r(wdh[:, ko, n * 512:(n + 1) * 512]),
                                     start=(kt == 0), stop=(kt == KT - 1))
                    nc.tensor.matmul(pB[n], lhsT=_r(bdec_pp[:, kt:kt + 1]),
                                     rhs=_r(wdh[:, ko, n * 512:(n + 1) * 512]),
                                     start=(kt == 0), stop=(kt == KT - 1))
        for n in range(2):
            nc.vector.tensor_copy(wfused[:, n * 512:(n + 1) * 512], pW[n])
            jt = junk([P, HID], BF16, "jstage2")
            jf = jt.bitcast(F32)
            nc.sync.dma_start(jf[0:1, :], io["b_dh"][n * 512:(n + 1) * 512][None, :])
            nc.vector.tensor_tensor(bfused_row[0:1, n * 512:(n + 1) * 512],
                                    pB[n][0:1, :], jf[0:1, :], ALU.add)

    # ---------------- per-chunk main pipeline ----------------
    _scope_stack = []
    c = 0

    def sc_(nm):
        if _scope_stack:
            prev_nm, prev_id = _scope_stack.pop()
            nc.leave_named_scope(prev_nm, prev_id, False)
        if nm is not None:
            full = f"c{c}_{nm}"
            sid, _ = nc.enter_named_scope(full, False)
            _scope_stack.append((full, sid))

    for c in range(NCHUNK):
        r0 = c * CH

        sc_("p1_transp")
        # --- phase 1a: load + transpose s and a ---
        a_rm = junk([P, MT, P], F32, "a_rm")
        nc.sync.dma_start(a_rm,
                          a_ap[r0:r0 + CH, :].rearrange("(rt p) f -> p rt f", p=P))

        sT = acts.tile([P, 4, CH], F32R, tag="sT", bufs=1, name="sT")
        aownT = acts.tile([ACTD, CH], F32R, tag="aownT", bufs=1, name="aownT")
        aothT = acts.tile([DEC_IN, CH], F32R, tag="aothT", bufs=1, name="aothT")
        for rh in range(2):
            s_rm = acts.tile([P, 2, OBS4], F32, tag="s_rm", bufs=1, name="s_rm")
            nc.sync.dma_start(
                s_rm,
                s_ap[r0 + rh * 256:r0 + (rh + 1) * 256, :].rearrange(
                    "(rt p) f -> p rt f", p=P))
            for ri in range(2):
                rt = rh * 2 + ri
                trp = pstr("trs")
                for ft in range(4):
                    nc.tensor.transpose(trp[:, ft * P:(ft + 1) * P],
                                        s_rm[:, ri, ft * P:(ft + 1) * P], identity)
                nc.vector.tensor_copy(sT[:, :, rt * P:(rt + 1) * P],
                                      trp.rearrange("p (ft x) -> p ft x", ft=4))
        for rt in range(MT):
            trp = pstr("tra")
            nc.tensor.transpose(trp[0:ACTD, 0:P], a_rm[:, rt, 0:ACTD], identity)
            nc.tensor.transpose(trp[0:DEC_IN, P:2 * P], a_rm[:, rt, ACTD:P], identity)
            nc.vector.tensor_copy(aownT[:, rt * P:(rt + 1) * P], trp[0:ACTD, 0:P])
            nc.vector.tensor_copy(aothT[:, rt * P:(rt + 1) * P],
                                  trp[0:DEC_IN, P:2 * P])

        sc_("p1b_enc")
        # --- phase 1b: enc_input^T = W_enc^T [s; a_own]^T + b  (no relu) ---
        if c == 0:
            wenc = wtile([P, 4, HID], "wenc")
            nc.sync.dma_start(
                wenc,
                io["W_enc_in"][0:512, :].rearrange(
                    "(ko p) f -> p ko f", p=P).bitcast(F32R))
        else:
            wenc = wenc_next  # DMA was hoisted to after the previous heads

        enc_inT = t8tile([P, KT, CH], "enc_inT")
        for m in range(KT):
            pm = psmm("pm_enc")
            for kt in range(4):
                nc.tensor.matmul(pm, lhsT=wenc[:, kt, m * P:(m + 1) * P],
                                 rhs=sT[:, kt, :], start=(kt == 0), stop=False)
            nc.tensor.matmul(pm, lhsT=wencr[:, m * P:(m + 1) * P],
                             rhs=aownT, start=False, stop=True)
            nc.scalar.activation(enc_inT[:, m, :], pm, AF.Identity,
                                 bias=b_enc_pp[:, m:m + 1])

        if c == 0:
            sc_("precompute")
            emit_precompute()

        sc_("p2a_eh")
        # --- phase 2a: encoder_h^T = relu(W_eh^T enc_input^T + b_eh) ---
        encHT = t8tile([P, KT, CH], "encHT")
        for mh in range(2):
            weh = wtile([P, KT, 512], "weh")
            nc.sync.dma_start(
                weh,
                io["W_eh"][:, mh * 512:(mh + 1) * 512].rearrange(
                    "(ko p) f -> p ko f", p=P).bitcast(F32R))
            for mi in range(4):
                m = mh * 4 + mi
                pm = psmm("pm_eh")
                for kt in range(KT):
                    nc.tensor.matmul(pm, lhsT=weh[:, kt, mi * P:(mi + 1) * P],
                                     rhs=enc_inT[:, kt, :],
                                     start=(kt == 0), stop=(kt == KT - 1))
                nc.scalar.activation(encHT[:, m, :], pm, AF.Relu,
                                     bias=b_eh_pp[:, m:m + 1])

        sc_("p2b_dh")
        # --- phase 2b: decoder_H = relu(a_others @ W_fused + b_fused), row-major ---
        DH = acts.tile([P, MT, HID], BF16, tag="dh", bufs=1, name="DH")
        for mt in range(MT):
            for n in range(2):
                pm = psmm("pm_dh")
                nc.tensor.matmul(pm, lhsT=aothT[:, mt * P:(mt + 1) * P],
                                 rhs=wfused[:, n * 512:(n + 1) * 512],
                                 start=True, stop=False)
                nc.tensor.matmul(pm, lhsT=ones_bf,
                                 rhs=bfused_row[0:1, n * 512:(n + 1) * 512],
                                 start=False, stop=True)
                nc.scalar.activation(DH[:, mt, n * 512:(n + 1) * 512], pm, AF.Relu)

        sc_("p3_heads")
        # --- phase 3: heads (row-major) + scores ---
        EH = acts.tile([P, MT, NH, HID], BF16, tag="eh", bufs=1, name="EH")
        scores = acts.tile([P, MT, NH], F32, tag="scores", bufs=2, name="scores")

        for h in range(NH):
            for n in range(2):
                whn = wtile([P, KT, 512], f"whn{h}_{n}")
                nc.sync.dma_start(
                    whn,
                    io["W_heads"][h][:, n * 512:(n + 1) * 512].rearrange(
                        "(ko p) f -> p ko f", p=P).bitcast(F32R))
                for mt in range(MT):
                    pm = psmm("pm_hd")
                    for kt in range(KT):
                        nc.tensor.matmul(
                            pm, lhsT=encHT[:, kt, mt * P:(mt + 1) * P],
                            rhs=whn[:, kt, :], start=(kt == 0), stop=False)
                    nc.tensor.matmul(pm, lhsT=ones_bf,
                                     rhs=bh_row[0:1, h, n * 512:(n + 1) * 512],
                                     start=False, stop=True)
                    nc.scalar.activation(EH[:, mt, h, n * 512:(n + 1) * 512],
                                         pm, AF.Relu)
            for mt in range(MT):
                # scores[:, mt, h] = rowsum(EH_h * DH): DVE multiply, then a
                # free-dim sum (alternating DVE/ACT to balance engine load).
                jt = junk([P, HID], BF16, "jsc")
                nc.vector.tensor_tensor(jt[:, :], EH[:, mt, h, :], DH[:, mt, :],
                                        ALU.mult)
                if mt % 2 == 0:
                    nc.scalar.activation(jt[:, :], jt[:, :], AF.Copy,
                                         accum_out=scores[:, mt, h:h + 1])
                else:
                    nc.vector.tensor_reduce(scores[:, mt, h:h + 1], jt[:, :],
                                            axis=AX.X, op=ALU.add)
        # prefetch next chunk's W_enc during the attention tail
        if c + 1 < NCHUNK:
            wenc_next = wtile([P, 4, HID], "wenc_n")
            nc.sync.dma_start(
                wenc_next,
                io["W_enc_in"][0:512, :].rearrange(
                    "(ko p) f -> p ko f", p=P).bitcast(F32R))

        sc_("p4_attn")
        # --- phase 4: softmax over heads (scores bounded -> no max-sub) and
        # context accumulation: ACT does the per-partition scale-copies while
        # DVE chains the adds, so the two engines pipeline the tail. ---
        attn = acts.tile([P, MT, NH], F32, tag="attn", bufs=2, name="attn")
        stats = acts.tile([P, MT, 2], F32, tag="stats", bufs=2, name="stats")
        ctx_t = acts.tile([P, MT, HID], BF16, tag="dh", bufs=1, name="ctx_t")
        for mt in range(MT):
            sumexp = stats[:, mt, 0:1]
            rsum = stats[:, mt, 1:2]
            nc.scalar.activation(attn[:, mt, :], scores[:, mt, :], AF.Exp,
                                 accum_out=sumexp)
            nc.vector.reciprocal(rsum, sumexp)
            nc.vector.tensor_scalar_mul(attn[:, mt, :], attn[:, mt, :], rsum)
        for mt in range(MT):
            nc.vector.tensor_scalar_mul(ctx_t[:, mt, :], EH[:, mt, 0, :],
                                        attn[:, mt, 0:1])
            for h in range(1, NH):
                nc.vector.scalar_tensor_tensor(
                    out=ctx_t[:, mt, :], in0=EH[:, mt, h, :],
                    scalar=attn[:, mt, h:h + 1],
                    in1=ctx_t[:, mt, :], op0=ALU.mult, op1=ALU.add)

        sc_("p5_ctxT")
        # --- phase 5: transpose ctx back to [feat, rows] (bf16 transposes) ---
        ctxT = t8tile([P, KT, CH], "ctxT")
        for mt in range(MT):
            for g in range(2):
                trp = pstr("trc", dtype=BF16)
                for ft in range(4):
                    nc.tensor.transpose(
                        trp[:, ft * P:(ft + 1) * P],
                        ctx_t[:, mt, (g * 4 + ft) * P:(g * 4 + ft + 1) * P],
                        identity_bf)
                nc.vector.tensor_copy(
                    ctxT[:, g * 4:(g + 1) * 4, mt * P:(mt + 1) * P],
                    trp.rearrange("p (ft x) -> p ft x", ft=4))

        sc_("p6_fc1")
        # --- phase 6: x1^T = relu(W1^T ctx^T + b1) ---
        x1T = t8tile([P, KT, CH], "x1T")
        for mh in range(2):
            w1 = wtile([P, KT, 512], "w1t")
            nc.sync.dma_start(
                w1,
                io["W1"][:, mh * 512:(mh + 1) * 512].rearrange(
                    "(ko p) f -> p ko f", p=P).bitcast(F32R))
            for mi in range(4):
                m = mh * 4 + mi
                pm = psmm("pm_fc1")
                for kt in range(KT):
                    nc.tensor.matmul(pm, lhsT=w1[:, kt, mi * P:(mi + 1) * P],
                                     rhs=ctxT[:, kt, :],
                                     start=(kt == 0), stop=(kt == KT - 1))
                nc.scalar.activation(x1T[:, m, :], pm, AF.Relu,
                                     bias=b1_pp[:, m:m + 1])

        sc_("p7_fc2")
        # --- phase 7: q^T = W2^T x1^T + b2  -> [1, CH] ---
        pq = psq("pq")
        for kt in range(KT):
            nc.tensor.matmul(pq, lhsT=W2sb[:, kt:kt + 1], rhs=x1T[:, kt, :],
                             start=(kt == 0), stop=(kt == KT - 1))
        q_rowT = acts.tile([1, CH], F32, tag="q_rowT", bufs=1, name="q_rowT")
        nc.scalar.activation(q_rowT[0:1, :], pq[0:1, :], AF.Identity,
                             bias=b2sb[0:1, 0:1])
        nc.sync.dma_start(q_ap[r0:r0 + CH, 0][None, :], q_rowT[0:1, :])
        sc_(None)


_NC_CACHE = None


def build():
    global _NC_CACHE
    if _NC_CACHE is not None:
        return _NC_CACHE
    nc = bacc.Bacc(trn_type="TRN2", target_bir_lowering=False, debug=False,
                   enable_asserts=False)
    io = {}
    io["s"] = nc.dram_tensor("s", [RPC, 512], F32, kind="ExternalInput").ap()
    io["a"] = nc.dram_tensor("a", [RPC, 128], F32, kind="ExternalInput").ap()
    shapes = {
        "W_enc_in": [544, HID], "b_enc_in": [HID],
        "W_dec_in": [DEC_IN, HID], "b_dec_in": [HID],
        "W_eh": [HID, HID], "b_eh": [HID],
        "W_heads": [NH, HID, HID], "b_heads": [NH, HID],
        "W_dh": [HID, HID], "b_dh": [HID],
        "W1": [HID, HID], "b1": [HID],
        "W2": [HID, 1], "b2": [1],
    }
    for name, shp in shapes.items():
        io[name] = nc.dram_tensor(name, shp, F32, kind="ExternalInput").ap()
    io["q"] = nc.dram_tensor("q", [RPC, 1], F32, kind="ExternalOutput").ap()

    from contextlib import ExitStack
    with tile.TileContext(nc) as tc, ExitStack() as ctx:
        _body(nc, tc, io, ctx)
    nc.compile()
    _NC_CACHE = nc
    return nc


def _shard_inputs(inputs):
    arrs = {k: np.ascontiguousarray(np.asarray(v, dtype=np.float32))
            for k, v in inputs.items()}
    in_maps = []
    for c in range(NCORES):
        m = {k: arrs[k] for k in WEIGHT_NAMES}
        m["s"] = np.ascontiguousarray(arrs["s"][c * RPC:(c + 1) * RPC])
        m["a"] = np.ascontiguousarray(arrs["a"][c * RPC:(c + 1) * RPC])
        in_maps.append(m)
    return in_maps


def run(inputs, trace=False):
    from concourse.bass_utils import run_bass_kernel_spmd
    nc = build()
    in_maps = _shard_inputs(inputs)
    res = run_bass_kernel_spmd(nc, in_maps, core_ids=list(range(NCORES)),
                               trace=trace)
    q = np.concatenate([r["q"] for r in res.results], axis=0)
    return np.ascontiguousarray(q.astype(np.float32)), res


def kernel(**inputs) -> np.ndarray:
    q, _ = run(inputs, trace=False)
    return q
